# revision 7
# baseline (speedup 1.0000x reference)
"""Trainium2 Bass kernel for nn_DQNModel (GGIN message passing DQN).

Sharding (graph-level data parallel over 8 NeuronCores):
  - Core d owns graphs [8d, 8d+8). Node rows live in a slab layout where
    graph g gets a fixed G_SLOT-row slot, so all per-graph slicing is static
    and the single SPMD program is shape-uniform across cores.
  - Edges are assigned to the core owning dst, sorted by dst cell (128-node
    ranges), padded per (cell, src-bucket) to whole 128-edge chunks.
  - Neighbor aggregation per layer: dma_gather of h[src] rows from a
    replicated slab table, then per-chunk one-hot matmuls on the tensor
    engine accumulate segment sums in PSUM (agg^T, transposed layout).
  - Dense part in transposed space: h_new^T = relu(W^T x^T + (W^T ctx + b)),
    with the per-graph context folded into the relu bias.
  - h is re-replicated between layers with an 8-core AllGather.
  - src node ids can exceed int16; edges are split into two buckets by slab
    row (< B0 / >= B0) gathered from offset table views.
"""

import os
import numpy as np
import ml_dtypes

import concourse.bass as bass
import concourse.mybir as mybir
import concourse.tile as tile
from concourse import bacc
from concourse.bass import ts, ds
from concourse.bass_utils import run_bass_kernel_spmd
from concourse.masks import make_identity

F32 = mybir.dt.float32
BF16 = mybir.dt.bfloat16
I16 = mybir.dt.int16

M = 8            # cores
NG = 64          # graphs
GPC = NG // M    # graphs per core
D = 128
NA = 40          # actions
W = 128          # node cell width (one-hot span)

TABLE_DT = os.environ.get("KERNEL_TABLE_DT", "bf16")
N_LAYERS = int(os.environ.get("KERNEL_LAYERS", "3"))
USE_AG = bool(int(os.environ.get("KERNEL_AG", "1")))
REPS = int(os.environ.get("KERNEL_REPS", "1"))
NO_GATHER = bool(int(os.environ.get("KERNEL_NO_GATHER", "0")))
N_QUEUES = int(os.environ.get("KERNEL_QUEUES", "4"))
SCRATCH = int(os.environ.get("KERNEL_SCRATCH", "16384"))
SUB_CH = int(os.environ.get("KERNEL_SUB_CH", "8"))  # chunks per gather sub-call


def cdiv(a, b):
    return (a + b - 1) // b


# ---------------------------------------------------------------- host plan


class Plan:
    pass


def build_plan(inputs, src, dst, graph_ids, init_feats, init_graph_ids,
               lead_feats, lead_graph_ids):
    p = Plan()
    np_tdt = ml_dtypes.bfloat16 if TABLE_DT == "bf16" else np.float32

    counts = np.bincount(graph_ids, minlength=NG)
    G_SLOT = max(1, cdiv(int(counts.max()), 128)) * 128
    N_SH = GPC * G_SLOT
    SLAB = M * N_SH
    assert SLAB <= 65534, SLAB
    B0 = SLAB // 2
    assert B0 <= 32767 and SLAB - B0 <= 32767

    gstart = np.zeros(NG + 1, np.int64)
    gstart[1:] = np.cumsum(counts)

    g_of_node = graph_ids.astype(np.int64)
    # local row within the owning core's shard
    owner_of_node = g_of_node // GPC
    r_local = ((g_of_node % GPC) * G_SLOT
               + (np.arange(len(graph_ids)) - gstart[g_of_node]))
    # slab layout: [half0: core0..7][half1: core0..7] so the inter-layer
    # replication can run as two AllGathers (one per half), and the int16
    # bucket boundary coincides with the half boundary.
    HALF = N_SH // 2
    half = r_local // HALF
    srow_of_node = half * (SLAB // 2) + owner_of_node * HALF + (r_local % HALF)

    p.G_SLOT, p.N_SH, p.SLAB, p.B0 = int(G_SLOT), int(N_SH), int(SLAB), int(B0)
    p.NT = N_SH // 128
    NCELL = p.NT
    p.CPS = G_SLOT // 128        # cells per graph slot

    p.HALF = int(HALF)

    # ---- edges
    eg = g_of_node[dst]
    owner = eg // GPC
    dst_rel = r_local[dst]
    srow = srow_of_node[src]
    bucket = (srow >= B0).astype(np.int64)
    idxval = (srow - bucket * B0).astype(np.int64)
    cell = dst_rel // W

    cnt = np.zeros((M, NCELL, 2), np.int64)
    np.add.at(cnt, (owner, cell, bucket), 1)
    CPC = cdiv(cnt.max(axis=0), 128)          # [NCELL, 2] chunks per cell/bucket
    p.CPC = CPC
    p.NCH_B = [int(CPC[:, 0].sum()), int(CPC[:, 1].sum())]
    p.NCHUNK = p.NCH_B[0] + p.NCH_B[1]

    # global chunk ids: bucket0 cells then bucket1 cells, cell-major
    chunk_base = np.zeros((NCELL, 2), np.int64)
    acc = 0
    for b in (0, 1):
        for c in range(NCELL):
            chunk_base[c, b] = acc
            acc += CPC[c, b]
    p.chunk_base = chunk_base
    p.bucket_chunk0 = [0, p.NCH_B[0]]         # first global chunk id per bucket

    p.idx_tiles = []
    p.dstcol = []
    for d in range(M):
        sel = owner == d
        e_cell = cell[sel]
        e_b = bucket[sel]
        e_idx = idxval[sel]
        e_dr = (dst_rel[sel] - e_cell * W).astype(np.float32)
        idxs_b = [np.zeros(max(1, p.NCH_B[b]) * 128, np.int16) for b in (0, 1)]
        dcol = np.full((128, max(1, p.NCHUNK)), -1.0, np.float32)
        for b in (0, 1):
            off = 0
            for c in range(NCELL):
                m = (e_cell == c) & (e_b == b)
                iv = e_idx[m]
                dr = e_dr[m]
                cap = int(CPC[c, b]) * 128
                assert len(iv) <= cap
                idxs_b[b][off:off + len(iv)] = iv
                for k in range(int(CPC[c, b])):
                    gch = int(chunk_base[c, b]) + k
                    seg = dr[k * 128:(k + 1) * 128]
                    dcol[: len(seg), gch] = seg
                off += cap
        tiles = []
        for b in (0, 1):
            arr = idxs_b[b].reshape(-1, 16).T          # [16, NCH*8]
            tiles.append(np.tile(arr, (8, 1)).astype(np.int16))
        p.idx_tiles.append(tiles)
        p.dstcol.append(dcol.astype(np.float32))

    # ---- gather groups: contiguous runs of graph slots
    p.NGRP = 4 if TABLE_DT == "bf16" else 8
    spg = max(1, cdiv(GPC, p.NGRP))            # slots per group
    p.groups = []
    s = 0
    while s < GPC:
        s1 = min(GPC, s + spg)
        p.groups.append((s * p.CPS, s1 * p.CPS))
        s = s1

    # max chunks in any (group, bucket) gather -> msg tile size
    p.MSG_CH = 1
    for (c0, c1) in p.groups:
        for b in (0, 1):
            nch = int(CPC[c0:c1, b].sum())
            p.MSG_CH = max(p.MSG_CH, nch)

    # ---- node features
    p.table0 = np.zeros((SLAB, D), np_tdt)
    p.table0[srow_of_node] = inputs.astype(np_tdt)
    p.rows0 = []
    p.gidcol = []
    for d in range(M):
        sel = owner_of_node == d
        r = np.zeros((N_SH, D), np.float32)
        r[r_local[sel]] = inputs[sel]
        p.rows0.append(r)
        g = np.full(N_SH, -1.0, np.float32)
        g[r_local[sel]] = (g_of_node[sel] % GPC).astype(np.float32)
        p.gidcol.append(np.ascontiguousarray(g.reshape(p.NT, 128).T))

    def shard_feats(feats, gids):
        cnts = np.bincount(gids, minlength=NG)
        gs = np.zeros(NG + 1, np.int64)
        gs[1:] = np.cumsum(cnts)
        starts = gs[::GPC]
        nloc = np.diff(starts)
        nt = max(1, cdiv(int(nloc.max()), 128))
        rows, gcols = [], []
        for d in range(M):
            r = np.zeros((nt * 128, D), np.float32)
            r[: nloc[d]] = feats[starts[d]:starts[d + 1]]
            g = np.full(nt * 128, -1.0, np.float32)
            g[: nloc[d]] = (gids[starts[d]:starts[d + 1]] % GPC)
            rows.append(r)
            gcols.append(np.ascontiguousarray(g.reshape(nt, 128).T))
        return nt, rows, gcols

    p.NIT, p.initrows, p.gidcol_init = shard_feats(init_feats, init_graph_ids)
    p.NLT, p.leadrows, p.gidcol_lead = shard_feats(lead_feats, lead_graph_ids)

    p.iota_oh = np.tile(np.arange(W, dtype=np.float32), (128, 1)).astype(np_tdt)
    p.iota_g = np.tile(np.arange(GPC, dtype=np.float32), (128, 1))
    return p


# ---------------------------------------------------------------- bass build


def build_bass(p):
    TDT = BF16 if TABLE_DT == "bf16" else F32
    nc = bacc.Bacc("TRN2", target_bir_lowering=False, debug=False,
                   num_swdge_queues=N_QUEUES,
                   dynamic_dma_scratch_size=SCRATCH)
    gather_q = [0]

    def next_q():
        q = gather_q[0]
        gather_q[0] = (q + 1) % N_QUEUES
        return q

    table0 = nc.dram_tensor("table0", [p.SLAB, D], TDT, kind="ExternalInput")
    rows0_d = nc.dram_tensor("rows0", [p.N_SH, D], F32, kind="ExternalInput")
    initrows_d = nc.dram_tensor("initrows", [p.NIT * 128, D], F32, kind="ExternalInput")
    leadrows_d = nc.dram_tensor("leadrows", [p.NLT * 128, D], F32, kind="ExternalInput")
    gidcol_d = nc.dram_tensor("gidcol", [128, p.NT], F32, kind="ExternalInput")
    gidcol_init_d = nc.dram_tensor("gidcol_init", [128, p.NIT], F32, kind="ExternalInput")
    gidcol_lead_d = nc.dram_tensor("gidcol_lead", [128, p.NLT], F32, kind="ExternalInput")
    dstcol_d = nc.dram_tensor("dstcol", [128, max(1, p.NCHUNK)], F32, kind="ExternalInput")
    idx_d = [
        nc.dram_tensor(f"idx{b}", [128, max(1, p.NCH_B[b]) * 8], I16,
                       kind="ExternalInput")
        for b in (0, 1)
    ]
    iota_oh_d = nc.dram_tensor("iota_oh", [128, W], TDT, kind="ExternalInput")
    iota_g_d = nc.dram_tensor("iota_g", [128, GPC], F32, kind="ExternalInput")
    Wl_d = [nc.dram_tensor(f"W{i}", [D, D], F32, kind="ExternalInput") for i in (1, 2, 3)]
    bl_d = [nc.dram_tensor(f"b{i}", [D, 1], F32, kind="ExternalInput") for i in (1, 2, 3)]
    fc1W_d = nc.dram_tensor("fc1W", [D, D], F32, kind="ExternalInput")
    fc1b_d = nc.dram_tensor("fc1b", [D, 1], F32, kind="ExternalInput")
    fc2W_d = nc.dram_tensor("fc2W", [D, NA], F32, kind="ExternalInput")
    fc2b_d = nc.dram_tensor("fc2b", [NA, 1], F32, kind="ExternalInput")
    q_d = nc.dram_tensor("q", [NA, GPC], F32, kind="ExternalOutput")

    agin = [nc.dram_tensor(f"agin{h}", [p.HALF, D], TDT) for h in (0, 1)]
    tables = [table0,
              nc.dram_tensor("table1", [p.SLAB, D], TDT, addr_space="Shared"),
              nc.dram_tensor("table2", [p.SLAB, D], TDT, addr_space="Shared")]

    NT = p.NT
    CPS = p.CPS

    with tile.TileContext(nc) as tc:
        with tc.tile_pool(name="big", bufs=1) as big, \
             tc.tile_pool(name="cst", bufs=1) as cst, \
             tc.tile_pool(name="msg", bufs=2) as msgp, \
             tc.tile_pool(name="oh", bufs=8) as ohp, \
             tc.tile_pool(name="sm", bufs=4) as smp, \
             tc.tile_pool(name="xw", bufs=3) as xwp, \
             tc.tile_pool(name="ld", bufs=2) as ldp, \
             tc.tile_pool(name="ps_agg", bufs=2, space="PSUM") as ps_agg, \
             tc.tile_pool(name="ps_dense", bufs=2, space="PSUM") as ps_dense, \
             tc.tile_pool(name="ps_tr", bufs=2, space="PSUM") as ps_tr, \
             tc.tile_pool(name="ps_pool", bufs=1, space="PSUM") as ps_pool:

            def load_const(dram, shape, dt, name):
                t = cst.tile(shape, dt, tag=name)
                nc.sync.dma_start(out=t[:], in_=dram[:])
                return t

            idx_sb = [
                load_const(idx_d[b], [128, max(1, p.NCH_B[b]) * 8], I16, f"idx{b}")
                for b in (0, 1)
            ]
            dstcol = load_const(dstcol_d, [128, max(1, p.NCHUNK)], F32, "dstcol")
            iota_oh = load_const(iota_oh_d, [128, W], TDT, "iota_oh")
            iota_g = load_const(iota_g_d, [128, GPC], F32, "iota_g")
            gidcol = load_const(gidcol_d, [128, NT], F32, "gidcol")
            gidcol_init = load_const(gidcol_init_d, [128, p.NIT], F32, "gidci")
            gidcol_lead = load_const(gidcol_lead_d, [128, p.NLT], F32, "gidcl")
            Wl = [load_const(Wl_d[i], [D, D], F32, f"W{i}") for i in range(3)]
            bl = [load_const(bl_d[i], [D, 1], F32, f"b{i}") for i in range(3)]
            fc1W = load_const(fc1W_d, [D, D], F32, "fc1W")
            fc1b = load_const(fc1b_d, [D, 1], F32, "fc1b")
            fc2W = load_const(fc2W_d, [D, NA], F32, "fc2W")
            fc2b = load_const(fc2b_d, [NA, 1], F32, "fc2b")

            ident = cst.tile([128, 128], F32, tag="ident")
            make_identity(nc, ident[:])

            rows = big.tile([128, NT, D], F32, tag="rows")
            hA = big.tile([128, p.N_SH], F32, tag="hA")
            hB = big.tile([128, p.N_SH], F32, tag="hB")

            for rep in range(REPS):
                nc.sync.dma_start(
                    out=rows[:], in_=rows0_d.rearrange("(t p) d -> p t d", p=128)
                )

                # ---- graph-pool of row tiles via one-hot matmuls
                def pool_tiles(get_tile, n_tiles, gcol_tile, psum_tile):
                    for j in range(n_tiles):
                        g = ohp.tile([128, GPC], F32, tag="goh")
                        nc.vector.tensor_scalar(
                            out=g[:], in0=iota_g[:], scalar1=gcol_tile[:, j:j + 1],
                            scalar2=None, op0=mybir.AluOpType.is_equal,
                        )
                        nc.tensor.matmul(
                            out=psum_tile[:], lhsT=get_tile(j), rhs=g[:],
                            start=(j == 0), stop=(j == n_tiles - 1),
                        )

                def pool_dram(dram, n_tiles, gcol_tile, psum_tile):
                    TL = 8
                    stages = {}
                    for j0 in range(0, n_tiles, TL):
                        j1 = min(n_tiles, j0 + TL)
                        st = ldp.tile([128, TL, D], F32, tag="stage")
                        nc.sync.dma_start(
                            out=st[:, : j1 - j0, :],
                            in_=dram.rearrange("(t p) d -> p t d", p=128)[:, j0:j1, :],
                        )
                        for j in range(j0, j1):
                            stages[j] = (st, j - j0)
                    pool_tiles(lambda j: stages[j][0][:, stages[j][1], :],
                               n_tiles, gcol_tile, psum_tile)

                # ---- static ctx = pool(init) + pool(lead)
                pool_init_ps = ps_pool.tile([128, GPC], F32, tag="pool")
                pool_dram(initrows_d, p.NIT, gidcol_init, pool_init_ps)
                ctx0 = cst.tile([128, GPC], F32, tag="ctx0")
                nc.vector.tensor_copy(out=ctx0[:], in_=pool_init_ps[:])
                pool_lead_ps = ps_pool.tile([128, GPC], F32, tag="pool")
                pool_dram(leadrows_d, p.NLT, gidcol_lead, pool_lead_ps)
                ctx_static = cst.tile([128, GPC], F32, tag="ctxs")
                nc.vector.tensor_tensor(
                    out=ctx_static[:], in0=pool_lead_ps[:], in1=ctx0[:],
                    op=mybir.AluOpType.add,
                )

                # ---- h^T for layer 1
                for t in range(NT):
                    tp = ps_tr.tile([128, 128], F32, tag="tr")
                    nc.tensor.transpose(out=tp[:], in_=rows[:, t, :], identity=ident[:])
                    nc.scalar.activation(
                        out=hA[:, ts(t, 128)], in_=tp[:],
                        func=mybir.ActivationFunctionType.Copy,
                    )

                # ---- layers
                hT, hN = hA, hB
                for li in range(N_LAYERS):
                    table = tables[li]
                    Wt, bt = Wl[li], bl[li]

                    pool_ps = ps_pool.tile([128, GPC], F32, tag="pool")
                    pool_tiles(lambda t: rows[:, t, :], NT, gidcol, pool_ps)
                    ctxg = smp.tile([128, GPC], F32, tag="ctxg")
                    nc.vector.tensor_tensor(
                        out=ctxg[:], in0=pool_ps[:], in1=ctx_static[:],
                        op=mybir.AluOpType.add,
                    )
                    ctxW_ps = ps_pool.tile([128, GPC], F32, tag="cw")
                    nc.tensor.matmul(out=ctxW_ps[:], lhsT=Wt[:], rhs=ctxg[:],
                                     start=True, stop=True)
                    biasg = smp.tile([128, GPC], F32, tag="biasg")
                    nc.vector.tensor_scalar(
                        out=biasg[:], in0=ctxW_ps[:], scalar1=bt[:, 0:1],
                        scalar2=None, op0=mybir.AluOpType.add,
                    )

                    xw_cur = [None]  # (tile, w0, wlen, slot)

                    def xw_for_cell(c):
                        slot, off = divmod(c, CPS)
                        wi = off // 4
                        w0 = slot * CPS + wi * 4
                        wlen = min(4, CPS - wi * 4)
                        if xw_cur[0] is None or xw_cur[0][1] != w0:
                            xw_t = xwp.tile([128, 4 * 128], F32, tag="xw")
                            xw_cur[0] = (xw_t, w0, wlen, slot)
                        return xw_cur[0]

                    def finish_cell(c, biasg=biasg, Wt=Wt, hN=hN):
                        t, w0, wlen, slot = xw_cur[0]
                        if c != w0 + wlen - 1:
                            return
                        n = wlen * 128
                        dp = ps_dense.tile([128, 4 * 128], F32, tag="dense")
                        nc.tensor.matmul(out=dp[:, :n], lhsT=Wt[:], rhs=t[:, :n],
                                         start=True, stop=True)
                        nc.scalar.activation(
                            out=hN[:, ds(w0 * 128, n)], in_=dp[:, :n],
                            func=mybir.ActivationFunctionType.Relu,
                            bias=biasg[:, slot:slot + 1],
                        )
                        xw_cur[0] = None

                    for (c0, c1) in p.groups:
                        mt = {}
                        for b in (0, 1):
                            rel0 = int(p.chunk_base[c0, b]) - p.bucket_chunk0[b]
                            rel1 = (int(p.chunk_base[c1 - 1, b] + p.CPC[c1 - 1, b])
                                    - p.bucket_chunk0[b])
                            nch = rel1 - rel0
                            if nch == 0:
                                mt[b] = None
                                continue
                            m = msgp.tile([128, p.MSG_CH, D], TDT, tag=f"msg{b}")
                            src_ap = (table[0:p.B0, :] if b == 0
                                      else table[p.B0:p.SLAB, :])
                            if NO_GATHER:
                                nc.vector.memset(m[:, 0, :], 0)
                                mt[b] = (m, rel0)
                                continue
                            nc.gpsimd.dma_gather(
                                out_ap=m[:, :nch, :],
                                in_ap=src_ap,
                                idxs_ap=idx_sb[b][:, rel0 * 8: rel1 * 8],
                                num_idxs=nch * 128,
                                num_idxs_reg=nch * 128,
                                elem_size=D,
                                single_packet=False,
                                queue_num=next_q(),
                            )
                            mt[b] = (m, rel0)

                        for c in range(c0, c1):
                            nchunks = int(p.CPC[c, 0] + p.CPC[c, 1])
                            xwt, q = xw_for_cell(c), None
                            xt, _, _, _ = xwt
                            col = ts(c - xwt[1], 128)
                            if nchunks == 0:
                                nc.vector.tensor_copy(
                                    out=xt[:, col], in_=hT[:, ts(c, 128)]
                                )
                                finish_cell(c)
                                continue
                            agg = ps_agg.tile([128, W], F32, tag="agg")
                            k = 0
                            for b in (0, 1):
                                if mt[b] is None:
                                    continue
                                m, rel0 = mt[b]
                                for j in range(int(p.CPC[c, b])):
                                    gch = int(p.chunk_base[c, b]) + j
                                    rel = gch - p.bucket_chunk0[b] - rel0
                                    oh = ohp.tile([128, W], TDT, tag="oh")
                                    nc.vector.tensor_scalar(
                                        out=oh[:], in0=iota_oh[:],
                                        scalar1=dstcol[:, gch:gch + 1],
                                        scalar2=None,
                                        op0=mybir.AluOpType.is_equal,
                                    )
                                    nc.tensor.matmul(
                                        out=agg[:], lhsT=m[:, rel, :], rhs=oh[:],
                                        start=(k == 0), stop=(k == nchunks - 1),
                                    )
                                    k += 1
                            tmp = smp.tile([128, W], F32, tag="tmp")
                            nc.scalar.activation(
                                out=tmp[:], in_=agg[:],
                                func=mybir.ActivationFunctionType.Copy,
                            )
                            nc.vector.tensor_tensor(
                                out=xt[:, col], in0=tmp[:], in1=hT[:, ts(c, 128)],
                                op=mybir.AluOpType.add,
                            )
                            finish_cell(c)

                    # hN -> rows (and replicate for next layer)
                    for t in range(NT):
                        tp = ps_tr.tile([128, 128], F32, tag="tr")
                        nc.tensor.transpose(
                            out=tp[:], in_=hN[:, ts(t, 128)], identity=ident[:]
                        )
                        nc.scalar.activation(
                            out=rows[:, t, :], in_=tp[:],
                            func=mybir.ActivationFunctionType.Copy,
                        )
                    if li < N_LAYERS - 1 and USE_AG:
                        HT = NT // 2
                        for h in (0, 1):
                            nc.gpsimd.dma_start(
                                out=agin[h].rearrange("(t p) d -> p t d", p=128),
                                in_=rows[:, h * HT:(h + 1) * HT, :],
                            )
                            nc.gpsimd.collective_compute(
                                "AllGather",
                                mybir.AluOpType.bypass,
                                ins=[agin[h][:]],
                                outs=[tables[li + 1][h * p.B0:(h + 1) * p.B0, :]],
                                replica_groups=[list(range(M))],
                            )
                    hT, hN = hN, hT

                # ---- head
                pool3 = ps_pool.tile([128, GPC], F32, tag="pool")
                pool_tiles(lambda t: rows[:, t, :], NT, gidcol, pool3)
                hg = smp.tile([128, GPC], F32, tag="hg")
                nc.vector.tensor_copy(out=hg[:], in_=pool3[:])
                h1_ps = ps_tr.tile([128, GPC], F32, tag="tr")
                nc.tensor.matmul(out=h1_ps[:], lhsT=fc1W[:], rhs=hg[:],
                                 start=True, stop=True)
                hfc = smp.tile([128, GPC], F32, tag="hfc")
                nc.scalar.activation(
                    out=hfc[:], in_=h1_ps[:],
                    func=mybir.ActivationFunctionType.Relu, bias=fc1b[:, 0:1],
                )
                q_ps = ps_tr.tile([NA, GPC], F32, tag="tr")
                nc.tensor.matmul(out=q_ps[:], lhsT=fc2W[:], rhs=hfc[:],
                                 start=True, stop=True)
                q_sb = smp.tile([NA, GPC], F32, tag="qsb")
                nc.vector.tensor_scalar(
                    out=q_sb[:], in0=q_ps[:], scalar1=fc2b[:, 0:1],
                    scalar2=None, op0=mybir.AluOpType.add,
                )
                nc.sync.dma_start(out=q_d[:], in_=q_sb[:])

    nc.compile()
    return nc


# ---------------------------------------------------------------- driver


_CACHE = {}


def _in_maps(p, inputs_np):
    maps = []
    for d in range(M):
        maps.append({
            "table0": np.asarray(p.table0),
            "rows0": p.rows0[d],
            "initrows": p.initrows[d],
            "leadrows": p.leadrows[d],
            "gidcol": p.gidcol[d],
            "gidcol_init": p.gidcol_init[d],
            "gidcol_lead": p.gidcol_lead[d],
            "dstcol": p.dstcol[d],
            "idx0": p.idx_tiles[d][0],
            "idx1": p.idx_tiles[d][1],
            "iota_oh": p.iota_oh,
            "iota_g": p.iota_g,
            "W1": inputs_np["W1"], "W2": inputs_np["W2"], "W3": inputs_np["W3"],
            "b1": inputs_np["b1"].reshape(D, 1),
            "b2": inputs_np["b2"].reshape(D, 1),
            "b3": inputs_np["b3"].reshape(D, 1),
            "fc1W": inputs_np["fc1_W"],
            "fc1b": inputs_np["fc1_b"].reshape(D, 1),
            "fc2W": inputs_np["fc2_W"],
            "fc2b": inputs_np["fc2_b"].reshape(NA, 1),
        })
    return maps


def _get_program(inputs_np):
    if "prog" not in _CACHE:
        p = build_plan(
            inputs_np["inputs"], inputs_np["src"], inputs_np["dst"],
            inputs_np["graph_ids"], inputs_np["init_feats"],
            inputs_np["init_graph_ids"], inputs_np["lead_feats"],
            inputs_np["lead_graph_ids"],
        )
        nc = build_bass(p)
        _CACHE["prog"] = (p, nc)
    return _CACHE["prog"]


def _run(inputs_np, trace=False):
    p, nc = _get_program(inputs_np)
    res = run_bass_kernel_spmd(nc, _in_maps(p, inputs_np), list(range(M)),
                               trace=trace)
    out = np.zeros((NG, NA), np.float32)
    for d in range(M):
        out[d * GPC:(d + 1) * GPC] = res.results[d]["q"].T
    return out, res


def kernel(**inputs):
    inputs_np = {k: np.asarray(v) for k, v in inputs.items()}
    out, _ = _run(inputs_np, trace=False)
    return out



# revision 13
# speedup vs baseline: 1.9871x; 1.9871x over previous
"""Trainium2 Bass kernel v2 for nn_DQNModel (GGIN message passing DQN).

Graph-level data parallel over 8 cores (core d owns graphs [8d, 8d+8)).
Key changes vs v1:
  - Quarter-major slab layout: srow = quarter*QB + owner*QSZ + r, so the
    inter-layer replication runs as 4 AllGathers (one per quarter) issued
    as soon as that quarter's rows are final -> overlaps compute.
  - Table rows bf16, but gather descriptors read 512B (two rows) via an
    overlapping access pattern (elem_step=128, elem_size=256): 512B
    descriptors are ~2x faster per descriptor than 256B on HW.
  - Gathers split into <=SUB_CH-chunk sub-calls, round-robined over 4
    SWDGE queues so the Pool engine never head-of-line blocks on a ring.
  - Tight chunking: per (gather-group, bucket) cells padded to the
    cross-core max count and chunked contiguously; chunks may straddle
    cells (one matmul per (chunk, cell) pair).
  - Pool one-hots precomputed on host; dst one-hots built 4-wide on DVE.
  - Agg matmuls in bf16; AllGather staging via a bf16 rows copy made on
    the Act engine and a HWDGE (sync) DMA, off the SWDGE/Pool path.
"""

import os
import numpy as np
import ml_dtypes

import bass_rust
import concourse.bass as bass
import concourse.mybir as mybir
import concourse.tile as tile
from concourse import bacc
from concourse.bass import ts, ds
from concourse.bass_utils import run_bass_kernel_spmd
from concourse.masks import make_identity

F32 = mybir.dt.float32
F32R = mybir.dt.float32r
BF16 = mybir.dt.bfloat16
I16 = mybir.dt.int16

M = 8            # cores
NG = 64          # graphs
GPC = NG // M    # graphs per core
D = 128
NA = 40          # actions
W = 128          # node cell width (one-hot span)

N_LAYERS = int(os.environ.get("K2_LAYERS", "3"))
USE_AG = bool(int(os.environ.get("K2_AG", "1")))
REPS = int(os.environ.get("K2_REPS", "1"))
N_QUEUES = int(os.environ.get("K2_QUEUES", "4"))
SUB_CH = int(os.environ.get("K2_SUB_CH", "8"))     # chunks per gather sub-call
NGG = int(os.environ.get("K2_NGG", "8"))           # gather groups (cells split)
NQTR = 4                                           # AllGather quarters
DENSE_DT = os.environ.get("K2_DENSE_DT", "f32")    # f32 | f32r


def cdiv(a, b):
    return (a + b - 1) // b


# ---------------------------------------------------------------- host plan


class Plan:
    pass


def build_plan(inputs, src, dst, graph_ids, init_feats, init_graph_ids,
               lead_feats, lead_graph_ids):
    p = Plan()

    counts = np.bincount(graph_ids, minlength=NG)
    G_SLOT = max(1, cdiv(int(counts.max()), 128)) * 128
    N_SH = GPC * G_SLOT
    SLAB = M * N_SH
    NT = N_SH // 128
    CPS = G_SLOT // 128
    assert NT % NQTR == 0 and (NT // NQTR) % (CPS) == 0, (NT, CPS)
    QSZ = N_SH // NQTR            # rows per core-quarter
    QB = M * QSZ                  # rows per slab-quarter
    B0 = 2 * QB                   # bucket boundary (int16 range)
    assert B0 <= 32767 and SLAB - B0 <= 32767
    TROWS = SLAB + 4              # two pad rows after each bucket

    gstart = np.zeros(NG + 1, np.int64)
    gstart[1:] = np.cumsum(counts)

    g_of_node = graph_ids.astype(np.int64)
    owner_of_node = g_of_node // GPC
    r_local = ((g_of_node % GPC) * G_SLOT
               + (np.arange(len(graph_ids)) - gstart[g_of_node]))
    quarter = r_local // QSZ
    srow_of_node = quarter * QB + owner_of_node * QSZ + (r_local % QSZ)

    p.G_SLOT, p.N_SH, p.SLAB, p.B0 = int(G_SLOT), int(N_SH), int(SLAB), int(B0)
    p.NT, p.CPS, p.QSZ, p.QB, p.TROWS = NT, CPS, int(QSZ), int(QB), TROWS
    CPG = NT // NGG               # cells per gather group

    # table row index: +2 pad rows after bucket0
    def trow(srow):
        return srow + 2 * (srow >= B0).astype(np.int64)

    # ---- edges
    eg = g_of_node[dst]
    owner = eg // GPC
    dst_rel = r_local[dst]
    cell = dst_rel // W
    col = dst_rel % W
    srow_e = srow_of_node[src]
    bkt = (srow_e >= B0).astype(np.int64)
    idxv = (srow_e - bkt * B0).astype(np.int64)

    # per (core, cell, bucket) counts -> cross-core max
    cnt = np.zeros((M, NT, 2), np.int64)
    np.add.at(cnt, (owner, cell, bkt), 1)
    cmax = cnt.max(axis=0)        # [NT, 2]

    # schedule: per (gg, b): cells of the group concatenated (padded to
    # cmax), chunked contiguously into 128-edge chunks.
    # mm plan: ordered per (gg, cell): list of (b, k_local, mm_id)
    p.NCH = np.zeros((NGG, 2), np.int64)       # chunks per (gg,b)
    cellpos = {}                               # (gg,b,c) -> start pos
    for gg in range(NGG):
        for b in (0, 1):
            pos = 0
            for c in range(gg * CPG, (gg + 1) * CPG):
                cellpos[(gg, b, c)] = pos
                pos += int(cmax[c, b])
            p.NCH[gg, b] = cdiv(pos, 128)
    p.MSG_CH = int(p.NCH.max())
    p.NCHB = [int(p.NCH[:, 0].sum()), int(p.NCH[:, 1].sum())]
    # chunk base per (gg,b) within that bucket's idx array
    chbase = np.zeros((NGG, 2), np.int64)
    for b in (0, 1):
        acc = 0
        for gg in range(NGG):
            chbase[gg, b] = acc
            acc += p.NCH[gg, b]
    p.chbase = chbase

    # mm plan (uniform across cores)
    mm_id = 0
    p.cell_mms = {}               # c -> list of (b, k_local, mm_id)
    for gg in range(NGG):
        for c in range(gg * CPG, (gg + 1) * CPG):
            mms = []
            for b in (0, 1):
                s = cellpos[(gg, b, c)]
                e = s + int(cmax[c, b])
                if e == s:
                    continue
                k0, k1 = s // 128, (e - 1) // 128
                for k in range(k0, k1 + 1):
                    mms.append((b, k, mm_id))
                    mm_id += 1
            p.cell_mms[c] = mms
    p.NMM = mm_id

    # per-core idx arrays and dstcol
    p.idx_tiles = []              # [core][b] -> [128, NCH_b*8] int16
    p.dstcol = []                 # [core] -> [128, NMM] bf16
    for d in range(M):
        sel = owner == d
        e_cell = cell[sel]
        e_b = bkt[sel]
        e_idx = idxv[sel]
        e_col = col[sel]
        idxs_b = [np.zeros(max(1, p.NCHB[b]) * 128, np.int16) for b in (0, 1)]
        dcol = np.full((128, max(1, p.NMM)), -1.0, np.float32)
        for gg in range(NGG):
            for c in range(gg * CPG, (gg + 1) * CPG):
                for b in (0, 1):
                    m = (e_cell == c) & (e_b == b)
                    iv = e_idx[m]
                    cv = e_col[m]
                    n = len(iv)
                    assert n <= cmax[c, b]
                    base = int(chbase[gg, b]) * 128 + cellpos[(gg, b, c)]
                    idxs_b[b][base:base + n] = iv
                    # mark dstcol for the mms covering this cell
                    for (bb, k, mm) in p.cell_mms[c]:
                        if bb != b:
                            continue
                        lo = k * 128
                        hi = lo + 128
                        # positions of this cell's real edges inside chunk k
                        s = cellpos[(gg, b, c)]
                        ps = np.arange(s, s + n)
                        ink = (ps >= lo) & (ps < hi)
                        dcol[ps[ink] - lo, mm] = cv[ink]
        p.idx_tiles.append([
            np.tile(a.reshape(-1, 16).T, (8, 1)).astype(np.int16)
            for a in idxs_b
        ])
        p.dstcol.append(dcol.astype(ml_dtypes.bfloat16))

    # ---- node features / pooling one-hots
    np_bf16 = ml_dtypes.bfloat16
    p.table0 = np.zeros((TROWS, D), np_bf16)
    p.table0[trow(srow_of_node)] = inputs.astype(np_bf16)
    p.rows0 = []
    p.pooloh = []
    for d in range(M):
        sel = owner_of_node == d
        r = np.zeros((N_SH, D), np.float32)
        r[r_local[sel]] = inputs[sel]
        p.rows0.append(r)
        g = np.full(N_SH, -1, np.int64)
        g[r_local[sel]] = g_of_node[sel] % GPC
        gt = g.reshape(NT, 128).T                       # [128, NT]
        oh = (gt[:, :, None] == np.arange(GPC)[None, None, :])
        p.pooloh.append(np.ascontiguousarray(oh.astype(np.float32)
                                             .reshape(128, NT * GPC)))

    def shard_feats(feats, gids):
        cnts = np.bincount(gids, minlength=NG)
        gs = np.zeros(NG + 1, np.int64)
        gs[1:] = np.cumsum(cnts)
        starts = gs[::GPC]
        nloc = np.diff(starts)
        nt = max(1, cdiv(int(nloc.max()), 128))
        rows, ohs = [], []
        for d in range(M):
            r = np.zeros((nt * 128, D), np.float32)
            r[: nloc[d]] = feats[starts[d]:starts[d + 1]]
            g = np.full(nt * 128, -1, np.int64)
            g[: nloc[d]] = gids[starts[d]:starts[d + 1]] % GPC
            gt = g.reshape(nt, 128).T
            oh = (gt[:, :, None] == np.arange(GPC)[None, None, :])
            rows.append(r)
            ohs.append(np.ascontiguousarray(oh.astype(np.float32)
                                            .reshape(128, nt * GPC)))
        return nt, rows, ohs

    p.NIT, p.initrows, p.pooloh_init = shard_feats(init_feats, init_graph_ids)
    p.NLT, p.leadrows, p.pooloh_lead = shard_feats(lead_feats, lead_graph_ids)

    p.iota_oh = np.tile(np.arange(W, dtype=np.float32), (128, 1)).astype(np_bf16)
    return p


# ---------------------------------------------------------------- bass build


def _overlap_view(t, row0, nrows):
    """AP over table rows [row0, row0+nrows) reading 256 elems (512B) per
    row index with 128-elem (256B) stride: descriptor i covers rows i, i+1.
    The underlying tensor must have one extra row past row0+nrows."""
    v = t[row0:row0 + nrows, :].copy()
    v.ap = bass_rust.VecI64Pair([(128, nrows), (1, 256)])
    return v


def build_bass(p, reps=REPS):
    nc = bacc.Bacc("TRN2", target_bir_lowering=False, debug=False,
                   num_swdge_queues=N_QUEUES)
    gq = [0]

    def next_q():
        q = gq[0]
        gq[0] = (q + 1) % N_QUEUES
        return q

    NT, CPS, QSZ, QB, B0, TROWS = p.NT, p.CPS, p.QSZ, p.QB, p.B0, p.TROWS
    CPQ = NT // NQTR              # cells per quarter (14)
    CPG = NT // NGG               # cells per gather group

    table0 = nc.dram_tensor("table0", [TROWS, D], BF16, kind="ExternalInput")
    rows0_d = nc.dram_tensor("rows0", [p.N_SH, D], F32, kind="ExternalInput")
    initrows_d = nc.dram_tensor("initrows", [p.NIT * 128, D], F32,
                                kind="ExternalInput")
    leadrows_d = nc.dram_tensor("leadrows", [p.NLT * 128, D], F32,
                                kind="ExternalInput")
    pooloh_d = nc.dram_tensor("pooloh", [128, NT * GPC], F32,
                              kind="ExternalInput")
    poolohi_d = nc.dram_tensor("poolohi", [128, p.NIT * GPC], F32,
                               kind="ExternalInput")
    poolohl_d = nc.dram_tensor("poolohl", [128, p.NLT * GPC], F32,
                               kind="ExternalInput")
    dstcol_d = nc.dram_tensor("dstcol", [128, max(1, p.NMM)], BF16,
                              kind="ExternalInput")
    idx_d = [
        nc.dram_tensor(f"idx{b}", [128, max(1, p.NCHB[b]) * 8], I16,
                       kind="ExternalInput")
        for b in (0, 1)
    ]
    iota_oh_d = nc.dram_tensor("iota_oh", [128, W], BF16, kind="ExternalInput")
    Wl_d = [nc.dram_tensor(f"W{i}", [D, D], F32, kind="ExternalInput")
            for i in (1, 2, 3)]
    bl_d = [nc.dram_tensor(f"b{i}", [D, 1], F32, kind="ExternalInput")
            for i in (1, 2, 3)]
    fc1W_d = nc.dram_tensor("fc1W", [D, D], F32, kind="ExternalInput")
    fc1b_d = nc.dram_tensor("fc1b", [D, 1], F32, kind="ExternalInput")
    fc2W_d = nc.dram_tensor("fc2W", [D, NA], F32, kind="ExternalInput")
    fc2b_d = nc.dram_tensor("fc2b", [NA, 1], F32, kind="ExternalInput")
    q_d = nc.dram_tensor("q", [NA, GPC], F32, kind="ExternalOutput")

    agin = [nc.dram_tensor(f"agin{qt}", [QSZ, D], BF16) for qt in range(NQTR)]
    tables = [table0,
              nc.dram_tensor("table1", [TROWS, D], BF16, addr_space="Shared"),
              nc.dram_tensor("table2", [TROWS, D], BF16, addr_space="Shared")]

    def bucket_view(table, b):
        return _overlap_view(table, b * (B0 + 2), B0 + 1)

    def table_q_rows(table, qt):
        r0 = qt * QB + (2 if qt >= 2 else 0)
        return table[r0:r0 + QB, :]

    with tile.TileContext(nc) as tc:
        with tc.tile_pool(name="big", bufs=1) as big, \
             tc.tile_pool(name="cst", bufs=1) as cst, \
             tc.tile_pool(name="msg", bufs=2) as msgp, \
             tc.tile_pool(name="oh", bufs=8) as ohp, \
             tc.tile_pool(name="sm", bufs=4) as smp, \
             tc.tile_pool(name="xw", bufs=3) as xwp, \
             tc.tile_pool(name="ld", bufs=2) as ldp, \
             tc.tile_pool(name="ps_agg", bufs=2, space="PSUM") as ps_agg, \
             tc.tile_pool(name="ps_dense", bufs=2, space="PSUM") as ps_dense, \
             tc.tile_pool(name="ps_tr", bufs=2, space="PSUM") as ps_tr, \
             tc.tile_pool(name="ps_pool", bufs=1, space="PSUM") as ps_pool:

            def load_const(dram, shape, dt, name):
                t = cst.tile(shape, dt, tag=name)
                nc.sync.dma_start(out=t[:], in_=dram[:])
                return t

            idx_sb = [
                load_const(idx_d[b], [128, max(1, p.NCHB[b]) * 8], I16,
                           f"idx{b}")
                for b in (0, 1)
            ]
            dstcol = load_const(dstcol_d, [128, max(1, p.NMM)], BF16, "dstcol")
            iota_oh = load_const(iota_oh_d, [128, W], BF16, "iota_oh")
            pooloh = load_const(pooloh_d, [128, NT, GPC], F32, "pooloh")
            poolohi = load_const(poolohi_d, [128, p.NIT, GPC], F32, "poolohi")
            poolohl = load_const(poolohl_d, [128, p.NLT, GPC], F32, "poolohl")
            Wl = [load_const(Wl_d[i], [D, D], F32, f"W{i}") for i in range(3)]
            bl = [load_const(bl_d[i], [D, 1], F32, f"b{i}") for i in range(3)]
            fc1W = load_const(fc1W_d, [D, D], F32, "fc1W")
            fc1b = load_const(fc1b_d, [D, 1], F32, "fc1b")
            fc2W = load_const(fc2W_d, [D, NA], F32, "fc2W")
            fc2b = load_const(fc2b_d, [NA, 1], F32, "fc2b")

            ident = cst.tile([128, 128], F32, tag="ident")
            make_identity(nc, ident[:])

            rows = big.tile([128, NT, D], F32, tag="rows")
            rows_bf = big.tile([128, NT, D], BF16, tag="rows_bf")
            hA = big.tile([128, p.N_SH], F32, tag="hA")
            hB = big.tile([128, p.N_SH], F32, tag="hB")

            def dense_cast(ap):
                if DENSE_DT == "f32r":
                    return ap.bitcast(F32R)
                return ap

            for rep in range(reps):
                nc.sync.dma_start(
                    out=rows[:], in_=rows0_d.rearrange("(t p) d -> p t d", p=128)
                )

                def pool_tiles(get_tile, n_tiles, oh_tile, psum_tile):
                    for j in range(n_tiles):
                        nc.tensor.matmul(
                            out=psum_tile[:], lhsT=get_tile(j),
                            rhs=oh_tile[:, j, :],
                            start=(j == 0), stop=(j == n_tiles - 1),
                        )

                def pool_dram(dram, n_tiles, oh_tile, psum_tile):
                    TL = 8
                    stages = {}
                    for j0 in range(0, n_tiles, TL):
                        j1 = min(n_tiles, j0 + TL)
                        st = ldp.tile([128, TL, D], F32, tag="stage")
                        nc.sync.dma_start(
                            out=st[:, : j1 - j0, :],
                            in_=dram.rearrange("(t p) d -> p t d", p=128)[:, j0:j1, :],
                        )
                        for j in range(j0, j1):
                            stages[j] = (st, j - j0)
                    pool_tiles(lambda j: stages[j][0][:, stages[j][1], :],
                               n_tiles, oh_tile, psum_tile)

                # ---- static ctx = pool(init) + pool(lead)
                pool_init_ps = ps_pool.tile([128, GPC], F32, tag="pool")
                pool_dram(initrows_d, p.NIT, poolohi, pool_init_ps)
                ctx0 = cst.tile([128, GPC], F32, tag="ctx0")
                nc.vector.tensor_copy(out=ctx0[:], in_=pool_init_ps[:])
                pool_lead_ps = ps_pool.tile([128, GPC], F32, tag="pool")
                pool_dram(leadrows_d, p.NLT, poolohl, pool_lead_ps)
                ctx_static = cst.tile([128, GPC], F32, tag="ctxs")
                nc.vector.tensor_tensor(
                    out=ctx_static[:], in0=pool_lead_ps[:], in1=ctx0[:],
                    op=mybir.AluOpType.add,
                )

                # ---- h^T for layer 1
                for t in range(NT):
                    tp = ps_tr.tile([128, 128], F32, tag="tr")
                    nc.tensor.transpose(out=tp[:], in_=rows[:, t, :],
                                        identity=ident[:])
                    nc.vector.tensor_copy(out=hA[:, ts(t, 128)], in_=tp[:])

                # ---- layers
                hT, hN = hA, hB
                for li in range(N_LAYERS):
                    table = tables[li]
                    Wt, bt = Wl[li], bl[li]

                    pool_ps = ps_pool.tile([128, GPC], F32, tag="pool")
                    pool_tiles(lambda t: rows[:, t, :], NT, pooloh, pool_ps)
                    ctxg = smp.tile([128, GPC], F32, tag="ctxg")
                    nc.vector.tensor_tensor(
                        out=ctxg[:], in0=pool_ps[:], in1=ctx_static[:],
                        op=mybir.AluOpType.add,
                    )
                    ctxW_ps = ps_pool.tile([128, GPC], F32, tag="cw")
                    nc.tensor.matmul(out=ctxW_ps[:], lhsT=Wt[:], rhs=ctxg[:],
                                     start=True, stop=True)
                    biasg = smp.tile([128, GPC], F32, tag="biasg")
                    nc.vector.tensor_scalar(
                        out=biasg[:], in0=ctxW_ps[:], scalar1=bt[:, 0:1],
                        scalar2=None, op0=mybir.AluOpType.add,
                    )

                    oh_cache = {}

                    def get_oh(mm, oh_cache=oh_cache):
                        bid = mm // 4
                        if bid not in oh_cache:
                            oh_cache.clear()
                            n = min(4, p.NMM - bid * 4)
                            t4 = ohp.tile([128, 4, W], BF16, tag="oh4")
                            nc.vector.tensor_tensor(
                                out=t4[:, :n, :],
                                in0=iota_oh[:, None, :].broadcast_to(
                                    [128, n, W]),
                                in1=dstcol[:, bid * 4: bid * 4 + n]
                                    .unsqueeze(2).broadcast_to([128, n, W]),
                                op=mybir.AluOpType.is_equal,
                            )
                            oh_cache[bid] = t4
                        return oh_cache[bid][:, mm % 4, :]

                    # dense/transpose block state (blocks within a quarter)
                    xw_cur = [None]  # (tile, c0, clen)

                    def xw_for_cell(c):
                        # blocks must stay within one graph slot (CPS cells)
                        # so the dense relu's per-graph bias column is right
                        slot, off = divmod(c, CPS)
                        bi = off // 4
                        c0 = slot * CPS + bi * 4
                        clen = min(4, CPS - bi * 4)
                        if xw_cur[0] is None or xw_cur[0][1] != c0:
                            xw_t = xwp.tile([128, 4 * 128], F32, tag="xw")
                            xw_cur[0] = (xw_t, c0, clen)
                        return xw_cur[0]

                    def finish_cell(c, biasg=biasg, Wt=Wt, hN=hN):
                        xt, c0, clen = xw_cur[0]
                        if c != c0 + clen - 1:
                            return
                        n = clen * 128
                        slot = c0 // CPS
                        dp = ps_dense.tile([128, 4 * 128], F32, tag="dense")
                        nc.tensor.matmul(out=dp[:, :n],
                                         lhsT=dense_cast(Wt[:]),
                                         rhs=dense_cast(xt[:, :n]),
                                         start=True, stop=True)
                        nc.scalar.activation(
                            out=hN[:, ds(c0 * 128, n)], in_=dp[:, :n],
                            func=mybir.ActivationFunctionType.Relu,
                            bias=biasg[:, slot:slot + 1],
                        )
                        # transpose the finished cells into rows (+bf16 copy
                        # for the AllGather staging, via Act to spread load)
                        for cc in range(c0, c0 + clen):
                            tp = ps_tr.tile([128, 128], F32, tag="tr")
                            nc.tensor.transpose(
                                out=tp[:], in_=hN[:, ts(cc, 128)],
                                identity=ident[:])
                            nc.vector.tensor_copy(out=rows[:, cc, :], in_=tp[:])
                            if li < N_LAYERS - 1 and USE_AG:
                                nc.scalar.activation(
                                    out=rows_bf[:, cc, :], in_=tp[:],
                                    func=mybir.ActivationFunctionType.Copy,
                                )
                        xw_cur[0] = None

                    for gg in range(NGG):
                        mt = {}
                        for b in (0, 1):
                            nch = int(p.NCH[gg, b])
                            if nch == 0:
                                mt[b] = None
                                continue
                            m = msgp.tile([128, p.MSG_CH, 256], BF16,
                                          tag=f"msg{b}")
                            src_ap = bucket_view(table, b)
                            base = int(p.chbase[gg, b])
                            for k0 in range(0, nch, SUB_CH):
                                k1 = min(nch, k0 + SUB_CH)
                                nc.gpsimd.dma_gather(
                                    out_ap=m[:, k0:k1, :],
                                    in_ap=src_ap,
                                    idxs_ap=idx_sb[b][:, (base + k0) * 8:
                                                      (base + k1) * 8],
                                    num_idxs=(k1 - k0) * 128,
                                    num_idxs_reg=(k1 - k0) * 128,
                                    elem_size=256,
                                    elem_step=128,
                                    single_packet=False,
                                    queue_num=next_q(),
                                )
                            mt[b] = m

                        for c in range(gg * CPG, (gg + 1) * CPG):
                            mms = p.cell_mms[c]
                            xt, c0, clen = xw_for_cell(c)
                            colsl = ts(c - c0, 128)
                            if not mms:
                                nc.vector.tensor_copy(
                                    out=xt[:, colsl], in_=hT[:, ts(c, 128)]
                                )
                                finish_cell(c)
                                continue
                            agg = ps_agg.tile([128, W], F32, tag="agg")
                            nmm = len(mms)
                            for j, (b, k, mm) in enumerate(mms):
                                nc.tensor.matmul(
                                    out=agg[:],
                                    lhsT=mt[b][:, k, 0:128],
                                    rhs=get_oh(mm),
                                    start=(j == 0), stop=(j == nmm - 1),
                                )
                            nc.vector.tensor_tensor(
                                out=xt[:, colsl], in0=agg[:],
                                in1=hT[:, ts(c, 128)],
                                op=mybir.AluOpType.add,
                            )
                            finish_cell(c)

                        # AllGather per quarter as soon as its rows are final
                        if li < N_LAYERS - 1 and USE_AG and gg % (NGG // NQTR) \
                                == (NGG // NQTR) - 1:
                            qt = gg // (NGG // NQTR)
                            nc.sync.dma_start(
                                out=agin[qt].rearrange("(t p) d -> p t d",
                                                       p=128),
                                in_=rows_bf[:, qt * CPQ:(qt + 1) * CPQ, :],
                            )
                            nc.gpsimd.collective_compute(
                                "AllGather",
                                mybir.AluOpType.bypass,
                                ins=[agin[qt][:]],
                                outs=[table_q_rows(tables[li + 1], qt)],
                                replica_groups=[list(range(M))],
                            )
                    hT, hN = hN, hT

                # ---- head
                pool3 = ps_pool.tile([128, GPC], F32, tag="pool")
                pool_tiles(lambda t: rows[:, t, :], NT, pooloh, pool3)
                hg = smp.tile([128, GPC], F32, tag="hg")
                nc.vector.tensor_copy(out=hg[:], in_=pool3[:])
                h1_ps = ps_tr.tile([128, GPC], F32, tag="tr")
                nc.tensor.matmul(out=h1_ps[:], lhsT=fc1W[:], rhs=hg[:],
                                 start=True, stop=True)
                hfc = smp.tile([128, GPC], F32, tag="hfc")
                nc.scalar.activation(
                    out=hfc[:], in_=h1_ps[:],
                    func=mybir.ActivationFunctionType.Relu, bias=fc1b[:, 0:1],
                )
                q_ps = ps_tr.tile([NA, GPC], F32, tag="tr")
                nc.tensor.matmul(out=q_ps[:], lhsT=fc2W[:], rhs=hfc[:],
                                 start=True, stop=True)
                q_sb = smp.tile([NA, GPC], F32, tag="qsb")
                nc.vector.tensor_scalar(
                    out=q_sb[:], in0=q_ps[:], scalar1=fc2b[:, 0:1],
                    scalar2=None, op0=mybir.AluOpType.add,
                )
                nc.sync.dma_start(out=q_d[:], in_=q_sb[:])

    nc.compile()
    return nc


# ---------------------------------------------------------------- driver


_CACHE = {}


def _in_maps(p, inputs_np):
    maps = []
    for d in range(M):
        maps.append({
            "table0": np.asarray(p.table0),
            "rows0": p.rows0[d],
            "initrows": p.initrows[d],
            "leadrows": p.leadrows[d],
            "pooloh": p.pooloh[d],
            "poolohi": p.pooloh_init[d],
            "poolohl": p.pooloh_lead[d],
            "dstcol": p.dstcol[d],
            "idx0": p.idx_tiles[d][0],
            "idx1": p.idx_tiles[d][1],
            "iota_oh": p.iota_oh,
            "W1": inputs_np["W1"], "W2": inputs_np["W2"], "W3": inputs_np["W3"],
            "b1": inputs_np["b1"].reshape(D, 1),
            "b2": inputs_np["b2"].reshape(D, 1),
            "b3": inputs_np["b3"].reshape(D, 1),
            "fc1W": inputs_np["fc1_W"],
            "fc1b": inputs_np["fc1_b"].reshape(D, 1),
            "fc2W": inputs_np["fc2_W"],
            "fc2b": inputs_np["fc2_b"].reshape(NA, 1),
        })
    return maps


def _get_program(inputs_np, reps=REPS):
    key = ("prog", reps)
    if key not in _CACHE:
        if "plan" not in _CACHE:
            _CACHE["plan"] = build_plan(
                inputs_np["inputs"], inputs_np["src"], inputs_np["dst"],
                inputs_np["graph_ids"], inputs_np["init_feats"],
                inputs_np["init_graph_ids"], inputs_np["lead_feats"],
                inputs_np["lead_graph_ids"],
            )
        p = _CACHE["plan"]
        nc = build_bass(p, reps=reps)
        _CACHE[key] = (p, nc)
    return _CACHE[key]


def _run(inputs_np, trace=False):
    p, nc = _get_program(inputs_np)
    res = run_bass_kernel_spmd(nc, _in_maps(p, inputs_np), list(range(M)),
                               trace=trace)
    out = np.zeros((NG, NA), np.float32)
    for d in range(M):
        out[d * GPC:(d + 1) * GPC] = res.results[d]["q"].T
    return out, res


def kernel(**inputs):
    inputs_np = {k: np.asarray(v) for k, v in inputs.items()}
    out, _ = _run(inputs_np, trace=False)
    return out


# revision 14
# speedup vs baseline: 3.6615x; 1.8426x over previous
"""Trainium2 Bass kernel v2 for nn_DQNModel (GGIN message passing DQN).

Graph-level data parallel over 8 cores (core d owns graphs [8d, 8d+8)).
Key changes vs v1:
  - Region-major slab layout: srow = region*QB + owner*QSZ + r, so the
    inter-layer replication runs as NQTR AllGathers issued as soon as
    that region's rows are final -> overlaps compute. NQTR=2 (one AG per
    int16 bucket) measures best: the first AG hides under the second
    half's compute and the collective count is minimal.
  - Table rows bf16, but gather descriptors read 512B (two rows) via an
    overlapping access pattern (elem_step=128, elem_size=256): 512B
    descriptors are ~2x faster per descriptor than 256B on HW.
  - Gathers split into <=SUB_CH-chunk sub-calls, round-robined over 4
    SWDGE queues so the Pool engine never head-of-line blocks on a ring.
  - Tight chunking: per (gather-group, bucket) cells padded to the
    cross-core max count and chunked contiguously; chunks may straddle
    cells (one matmul per (chunk, cell) pair).
  - Pool one-hots precomputed on host; dst one-hots built 4-wide on DVE.
  - Agg matmuls in bf16; AllGather staging via a bf16 rows copy made on
    the Act engine and a HWDGE (sync) DMA, off the SWDGE/Pool path.
"""

import os
import numpy as np
import ml_dtypes

import bass_rust
import concourse.bass as bass
import concourse.mybir as mybir
import concourse.tile as tile
from concourse import bacc
from concourse.bass import ts, ds
from concourse.bass_utils import run_bass_kernel_spmd
from concourse.masks import make_identity

F32 = mybir.dt.float32
F32R = mybir.dt.float32r
BF16 = mybir.dt.bfloat16
I16 = mybir.dt.int16

M = 8            # cores
NG = 64          # graphs
GPC = NG // M    # graphs per core
D = 128
NA = 40          # actions
W = 128          # node cell width (one-hot span)

N_LAYERS = int(os.environ.get("K2_LAYERS", "3"))
USE_AG = bool(int(os.environ.get("K2_AG", "1")))
REPS = int(os.environ.get("K2_REPS", "1"))
N_QUEUES = int(os.environ.get("K2_QUEUES", "4"))
SUB_CH = int(os.environ.get("K2_SUB_CH", "8"))     # chunks per gather sub-call
NGG = int(os.environ.get("K2_NGG", "8"))           # gather groups (cells split)
NQTR = int(os.environ.get("K2_NQTR", "2"))         # AllGather regions per layer
DENSE_DT = os.environ.get("K2_DENSE_DT", "f32")    # f32 | f32r


def cdiv(a, b):
    return (a + b - 1) // b


# ---------------------------------------------------------------- host plan


class Plan:
    pass


def build_plan(inputs, src, dst, graph_ids, init_feats, init_graph_ids,
               lead_feats, lead_graph_ids):
    p = Plan()

    counts = np.bincount(graph_ids, minlength=NG)
    G_SLOT = max(1, cdiv(int(counts.max()), 128)) * 128
    N_SH = GPC * G_SLOT
    SLAB = M * N_SH
    NT = N_SH // 128
    CPS = G_SLOT // 128
    assert NT % NQTR == 0 and (NT // NQTR) % (CPS) == 0, (NT, CPS)
    QSZ = N_SH // NQTR            # rows per core-region
    QB = M * QSZ                  # rows per slab-region
    B0 = SLAB // 2                # bucket boundary (int16 range)
    assert B0 <= 32767 and SLAB - B0 <= 32767
    TROWS = SLAB + 4              # two pad rows after each bucket

    gstart = np.zeros(NG + 1, np.int64)
    gstart[1:] = np.cumsum(counts)

    g_of_node = graph_ids.astype(np.int64)
    owner_of_node = g_of_node // GPC
    r_local = ((g_of_node % GPC) * G_SLOT
               + (np.arange(len(graph_ids)) - gstart[g_of_node]))
    quarter = r_local // QSZ
    srow_of_node = quarter * QB + owner_of_node * QSZ + (r_local % QSZ)

    p.G_SLOT, p.N_SH, p.SLAB, p.B0 = int(G_SLOT), int(N_SH), int(SLAB), int(B0)
    p.NT, p.CPS, p.QSZ, p.QB, p.TROWS = NT, CPS, int(QSZ), int(QB), TROWS
    CPG = NT // NGG               # cells per gather group

    # table row index: +2 pad rows after bucket0
    def trow(srow):
        return srow + 2 * (srow >= B0).astype(np.int64)

    # ---- edges
    eg = g_of_node[dst]
    owner = eg // GPC
    dst_rel = r_local[dst]
    cell = dst_rel // W
    col = dst_rel % W
    srow_e = srow_of_node[src]
    bkt = (srow_e >= B0).astype(np.int64)
    idxv = (srow_e - bkt * B0).astype(np.int64)

    # per (core, cell, bucket) counts -> cross-core max
    cnt = np.zeros((M, NT, 2), np.int64)
    np.add.at(cnt, (owner, cell, bkt), 1)
    cmax = cnt.max(axis=0)        # [NT, 2]

    # schedule: per (gg, b): cells of the group concatenated (padded to
    # cmax), chunked contiguously into 128-edge chunks.
    # mm plan: ordered per (gg, cell): list of (b, k_local, mm_id)
    p.NCH = np.zeros((NGG, 2), np.int64)       # chunks per (gg,b)
    cellpos = {}                               # (gg,b,c) -> start pos
    for gg in range(NGG):
        for b in (0, 1):
            pos = 0
            for c in range(gg * CPG, (gg + 1) * CPG):
                cellpos[(gg, b, c)] = pos
                pos += int(cmax[c, b])
            p.NCH[gg, b] = cdiv(pos, 128)
    p.MSG_CH = int(p.NCH.max())
    p.NCHB = [int(p.NCH[:, 0].sum()), int(p.NCH[:, 1].sum())]
    # chunk base per (gg,b) within that bucket's idx array
    chbase = np.zeros((NGG, 2), np.int64)
    for b in (0, 1):
        acc = 0
        for gg in range(NGG):
            chbase[gg, b] = acc
            acc += p.NCH[gg, b]
    p.chbase = chbase

    # mm plan (uniform across cores)
    mm_id = 0
    p.cell_mms = {}               # c -> list of (b, k_local, mm_id)
    for gg in range(NGG):
        for c in range(gg * CPG, (gg + 1) * CPG):
            mms = []
            for b in (0, 1):
                s = cellpos[(gg, b, c)]
                e = s + int(cmax[c, b])
                if e == s:
                    continue
                k0, k1 = s // 128, (e - 1) // 128
                for k in range(k0, k1 + 1):
                    mms.append((b, k, mm_id))
                    mm_id += 1
            p.cell_mms[c] = mms
    p.NMM = mm_id

    # per-core idx arrays and dstcol
    p.idx_tiles = []              # [core][b] -> [128, NCH_b*8] int16
    p.dstcol = []                 # [core] -> [128, NMM] bf16
    for d in range(M):
        sel = owner == d
        e_cell = cell[sel]
        e_b = bkt[sel]
        e_idx = idxv[sel]
        e_col = col[sel]
        idxs_b = [np.zeros(max(1, p.NCHB[b]) * 128, np.int16) for b in (0, 1)]
        dcol = np.full((128, max(1, p.NMM)), -1.0, np.float32)
        for gg in range(NGG):
            for c in range(gg * CPG, (gg + 1) * CPG):
                for b in (0, 1):
                    m = (e_cell == c) & (e_b == b)
                    iv = e_idx[m]
                    cv = e_col[m]
                    n = len(iv)
                    assert n <= cmax[c, b]
                    base = int(chbase[gg, b]) * 128 + cellpos[(gg, b, c)]
                    idxs_b[b][base:base + n] = iv
                    # mark dstcol for the mms covering this cell
                    for (bb, k, mm) in p.cell_mms[c]:
                        if bb != b:
                            continue
                        lo = k * 128
                        hi = lo + 128
                        # positions of this cell's real edges inside chunk k
                        s = cellpos[(gg, b, c)]
                        ps = np.arange(s, s + n)
                        ink = (ps >= lo) & (ps < hi)
                        dcol[ps[ink] - lo, mm] = cv[ink]
        p.idx_tiles.append([
            np.tile(a.reshape(-1, 16).T, (8, 1)).astype(np.int16)
            for a in idxs_b
        ])
        p.dstcol.append(dcol.astype(ml_dtypes.bfloat16))

    # ---- node features / pooling one-hots
    np_bf16 = ml_dtypes.bfloat16
    p.table0 = np.zeros((TROWS, D), np_bf16)
    p.table0[trow(srow_of_node)] = inputs.astype(np_bf16)
    p.rows0 = []
    p.pooloh = []
    for d in range(M):
        sel = owner_of_node == d
        r = np.zeros((N_SH, D), np.float32)
        r[r_local[sel]] = inputs[sel]
        p.rows0.append(r)
        g = np.full(N_SH, -1, np.int64)
        g[r_local[sel]] = g_of_node[sel] % GPC
        gt = g.reshape(NT, 128).T                       # [128, NT]
        oh = (gt[:, :, None] == np.arange(GPC)[None, None, :])
        p.pooloh.append(np.ascontiguousarray(oh.astype(np.float32)
                                             .reshape(128, NT * GPC)))

    def shard_feats(feats, gids):
        cnts = np.bincount(gids, minlength=NG)
        gs = np.zeros(NG + 1, np.int64)
        gs[1:] = np.cumsum(cnts)
        starts = gs[::GPC]
        nloc = np.diff(starts)
        nt = max(1, cdiv(int(nloc.max()), 128))
        rows, ohs = [], []
        for d in range(M):
            r = np.zeros((nt * 128, D), np.float32)
            r[: nloc[d]] = feats[starts[d]:starts[d + 1]]
            g = np.full(nt * 128, -1, np.int64)
            g[: nloc[d]] = gids[starts[d]:starts[d + 1]] % GPC
            gt = g.reshape(nt, 128).T
            oh = (gt[:, :, None] == np.arange(GPC)[None, None, :])
            rows.append(r)
            ohs.append(np.ascontiguousarray(oh.astype(np.float32)
                                            .reshape(128, nt * GPC)))
        return nt, rows, ohs

    p.NIT, p.initrows, p.pooloh_init = shard_feats(init_feats, init_graph_ids)
    p.NLT, p.leadrows, p.pooloh_lead = shard_feats(lead_feats, lead_graph_ids)

    p.iota_oh = np.tile(np.arange(W, dtype=np.float32), (128, 1)).astype(np_bf16)
    return p


# ---------------------------------------------------------------- bass build


def _overlap_view(t, row0, nrows):
    """AP over table rows [row0, row0+nrows) reading 256 elems (512B) per
    row index with 128-elem (256B) stride: descriptor i covers rows i, i+1.
    The underlying tensor must have one extra row past row0+nrows."""
    v = t[row0:row0 + nrows, :].copy()
    v.ap = bass_rust.VecI64Pair([(128, nrows), (1, 256)])
    return v


def build_bass(p, reps=REPS):
    nc = bacc.Bacc("TRN2", target_bir_lowering=False, debug=False,
                   num_swdge_queues=N_QUEUES)
    gq = [0]

    def next_q():
        q = gq[0]
        gq[0] = (q + 1) % N_QUEUES
        return q

    NT, CPS, QSZ, QB, B0, TROWS = p.NT, p.CPS, p.QSZ, p.QB, p.B0, p.TROWS
    CPQ = NT // NQTR              # cells per quarter (14)
    CPG = NT // NGG               # cells per gather group

    table0 = nc.dram_tensor("table0", [TROWS, D], BF16, kind="ExternalInput")
    rows0_d = nc.dram_tensor("rows0", [p.N_SH, D], F32, kind="ExternalInput")
    initrows_d = nc.dram_tensor("initrows", [p.NIT * 128, D], F32,
                                kind="ExternalInput")
    leadrows_d = nc.dram_tensor("leadrows", [p.NLT * 128, D], F32,
                                kind="ExternalInput")
    pooloh_d = nc.dram_tensor("pooloh", [128, NT * GPC], F32,
                              kind="ExternalInput")
    poolohi_d = nc.dram_tensor("poolohi", [128, p.NIT * GPC], F32,
                               kind="ExternalInput")
    poolohl_d = nc.dram_tensor("poolohl", [128, p.NLT * GPC], F32,
                               kind="ExternalInput")
    dstcol_d = nc.dram_tensor("dstcol", [128, max(1, p.NMM)], BF16,
                              kind="ExternalInput")
    idx_d = [
        nc.dram_tensor(f"idx{b}", [128, max(1, p.NCHB[b]) * 8], I16,
                       kind="ExternalInput")
        for b in (0, 1)
    ]
    iota_oh_d = nc.dram_tensor("iota_oh", [128, W], BF16, kind="ExternalInput")
    Wl_d = [nc.dram_tensor(f"W{i}", [D, D], F32, kind="ExternalInput")
            for i in (1, 2, 3)]
    bl_d = [nc.dram_tensor(f"b{i}", [D, 1], F32, kind="ExternalInput")
            for i in (1, 2, 3)]
    fc1W_d = nc.dram_tensor("fc1W", [D, D], F32, kind="ExternalInput")
    fc1b_d = nc.dram_tensor("fc1b", [D, 1], F32, kind="ExternalInput")
    fc2W_d = nc.dram_tensor("fc2W", [D, NA], F32, kind="ExternalInput")
    fc2b_d = nc.dram_tensor("fc2b", [NA, 1], F32, kind="ExternalInput")
    q_d = nc.dram_tensor("q", [NA, GPC], F32, kind="ExternalOutput")

    agin = [nc.dram_tensor(f"agin{qt}", [QSZ, D], BF16) for qt in range(NQTR)]
    tables = [table0,
              nc.dram_tensor("table1", [TROWS, D], BF16, addr_space="Shared"),
              nc.dram_tensor("table2", [TROWS, D], BF16, addr_space="Shared")]

    def bucket_view(table, b):
        return _overlap_view(table, b * (B0 + 2), B0 + 1)

    def table_q_rows(table, qt):
        r0 = qt * QB + (2 if qt >= NQTR // 2 else 0)
        return table[r0:r0 + QB, :]

    with tile.TileContext(nc) as tc:
        with tc.tile_pool(name="big", bufs=1) as big, \
             tc.tile_pool(name="cst", bufs=1) as cst, \
             tc.tile_pool(name="msg", bufs=2) as msgp, \
             tc.tile_pool(name="oh", bufs=8) as ohp, \
             tc.tile_pool(name="sm", bufs=4) as smp, \
             tc.tile_pool(name="xw", bufs=3) as xwp, \
             tc.tile_pool(name="ld", bufs=2) as ldp, \
             tc.tile_pool(name="ps_agg", bufs=2, space="PSUM") as ps_agg, \
             tc.tile_pool(name="ps_dense", bufs=2, space="PSUM") as ps_dense, \
             tc.tile_pool(name="ps_tr", bufs=2, space="PSUM") as ps_tr, \
             tc.tile_pool(name="ps_pool", bufs=1, space="PSUM") as ps_pool:

            def load_const(dram, shape, dt, name):
                t = cst.tile(shape, dt, tag=name)
                nc.sync.dma_start(out=t[:], in_=dram[:])
                return t

            idx_sb = [
                load_const(idx_d[b], [128, max(1, p.NCHB[b]) * 8], I16,
                           f"idx{b}")
                for b in (0, 1)
            ]
            dstcol = load_const(dstcol_d, [128, max(1, p.NMM)], BF16, "dstcol")
            iota_oh = load_const(iota_oh_d, [128, W], BF16, "iota_oh")
            pooloh = load_const(pooloh_d, [128, NT, GPC], F32, "pooloh")
            poolohi = load_const(poolohi_d, [128, p.NIT, GPC], F32, "poolohi")
            poolohl = load_const(poolohl_d, [128, p.NLT, GPC], F32, "poolohl")
            Wl = [load_const(Wl_d[i], [D, D], F32, f"W{i}") for i in range(3)]
            bl = [load_const(bl_d[i], [D, 1], F32, f"b{i}") for i in range(3)]
            fc1W = load_const(fc1W_d, [D, D], F32, "fc1W")
            fc1b = load_const(fc1b_d, [D, 1], F32, "fc1b")
            fc2W = load_const(fc2W_d, [D, NA], F32, "fc2W")
            fc2b = load_const(fc2b_d, [NA, 1], F32, "fc2b")

            ident = cst.tile([128, 128], F32, tag="ident")
            make_identity(nc, ident[:])

            rows = big.tile([128, NT, D], F32, tag="rows")
            rows_bf = big.tile([128, NT, D], BF16, tag="rows_bf")
            hA = big.tile([128, p.N_SH], F32, tag="hA")
            hB = big.tile([128, p.N_SH], F32, tag="hB")

            def dense_cast(ap):
                if DENSE_DT == "f32r":
                    return ap.bitcast(F32R)
                return ap

            for rep in range(reps):
                nc.sync.dma_start(
                    out=rows[:], in_=rows0_d.rearrange("(t p) d -> p t d", p=128)
                )

                def pool_tiles(get_tile, n_tiles, oh_tile, psum_tile):
                    for j in range(n_tiles):
                        nc.tensor.matmul(
                            out=psum_tile[:], lhsT=get_tile(j),
                            rhs=oh_tile[:, j, :],
                            start=(j == 0), stop=(j == n_tiles - 1),
                        )

                def pool_dram(dram, n_tiles, oh_tile, psum_tile):
                    TL = 8
                    stages = {}
                    for j0 in range(0, n_tiles, TL):
                        j1 = min(n_tiles, j0 + TL)
                        st = ldp.tile([128, TL, D], F32, tag="stage")
                        nc.sync.dma_start(
                            out=st[:, : j1 - j0, :],
                            in_=dram.rearrange("(t p) d -> p t d", p=128)[:, j0:j1, :],
                        )
                        for j in range(j0, j1):
                            stages[j] = (st, j - j0)
                    pool_tiles(lambda j: stages[j][0][:, stages[j][1], :],
                               n_tiles, oh_tile, psum_tile)

                # ---- static ctx = pool(init) + pool(lead)
                pool_init_ps = ps_pool.tile([128, GPC], F32, tag="pool")
                pool_dram(initrows_d, p.NIT, poolohi, pool_init_ps)
                ctx0 = cst.tile([128, GPC], F32, tag="ctx0")
                nc.vector.tensor_copy(out=ctx0[:], in_=pool_init_ps[:])
                pool_lead_ps = ps_pool.tile([128, GPC], F32, tag="pool")
                pool_dram(leadrows_d, p.NLT, poolohl, pool_lead_ps)
                ctx_static = cst.tile([128, GPC], F32, tag="ctxs")
                nc.vector.tensor_tensor(
                    out=ctx_static[:], in0=pool_lead_ps[:], in1=ctx0[:],
                    op=mybir.AluOpType.add,
                )

                # ---- h^T for layer 1
                for t in range(NT):
                    tp = ps_tr.tile([128, 128], F32, tag="tr")
                    nc.tensor.transpose(out=tp[:], in_=rows[:, t, :],
                                        identity=ident[:])
                    nc.vector.tensor_copy(out=hA[:, ts(t, 128)], in_=tp[:])

                # ---- layers
                hT, hN = hA, hB
                for li in range(N_LAYERS):
                    table = tables[li]
                    Wt, bt = Wl[li], bl[li]

                    pool_ps = ps_pool.tile([128, GPC], F32, tag="pool")
                    pool_tiles(lambda t: rows[:, t, :], NT, pooloh, pool_ps)
                    ctxg = smp.tile([128, GPC], F32, tag="ctxg")
                    nc.vector.tensor_tensor(
                        out=ctxg[:], in0=pool_ps[:], in1=ctx_static[:],
                        op=mybir.AluOpType.add,
                    )
                    ctxW_ps = ps_pool.tile([128, GPC], F32, tag="cw")
                    nc.tensor.matmul(out=ctxW_ps[:], lhsT=Wt[:], rhs=ctxg[:],
                                     start=True, stop=True)
                    biasg = smp.tile([128, GPC], F32, tag="biasg")
                    nc.vector.tensor_scalar(
                        out=biasg[:], in0=ctxW_ps[:], scalar1=bt[:, 0:1],
                        scalar2=None, op0=mybir.AluOpType.add,
                    )

                    oh_cache = {}

                    def get_oh(mm, oh_cache=oh_cache):
                        bid = mm // 4
                        if bid not in oh_cache:
                            oh_cache.clear()
                            n = min(4, p.NMM - bid * 4)
                            t4 = ohp.tile([128, 4, W], BF16, tag="oh4")
                            nc.vector.tensor_tensor(
                                out=t4[:, :n, :],
                                in0=iota_oh[:, None, :].broadcast_to(
                                    [128, n, W]),
                                in1=dstcol[:, bid * 4: bid * 4 + n]
                                    .unsqueeze(2).broadcast_to([128, n, W]),
                                op=mybir.AluOpType.is_equal,
                            )
                            oh_cache[bid] = t4
                        return oh_cache[bid][:, mm % 4, :]

                    # dense/transpose block state (blocks within a quarter)
                    xw_cur = [None]  # (tile, c0, clen)

                    def xw_for_cell(c):
                        # blocks must stay within one graph slot (CPS cells)
                        # so the dense relu's per-graph bias column is right
                        slot, off = divmod(c, CPS)
                        bi = off // 4
                        c0 = slot * CPS + bi * 4
                        clen = min(4, CPS - bi * 4)
                        if xw_cur[0] is None or xw_cur[0][1] != c0:
                            xw_t = xwp.tile([128, 4 * 128], F32, tag="xw")
                            xw_cur[0] = (xw_t, c0, clen)
                        return xw_cur[0]

                    def finish_cell(c, biasg=biasg, Wt=Wt, hN=hN):
                        xt, c0, clen = xw_cur[0]
                        if c != c0 + clen - 1:
                            return
                        n = clen * 128
                        slot = c0 // CPS
                        dp = ps_dense.tile([128, 4 * 128], F32, tag="dense")
                        nc.tensor.matmul(out=dp[:, :n],
                                         lhsT=dense_cast(Wt[:]),
                                         rhs=dense_cast(xt[:, :n]),
                                         start=True, stop=True)
                        nc.scalar.activation(
                            out=hN[:, ds(c0 * 128, n)], in_=dp[:, :n],
                            func=mybir.ActivationFunctionType.Relu,
                            bias=biasg[:, slot:slot + 1],
                        )
                        # transpose the finished cells into rows (+bf16 copy
                        # for the AllGather staging, via Act to spread load)
                        for cc in range(c0, c0 + clen):
                            tp = ps_tr.tile([128, 128], F32, tag="tr")
                            nc.tensor.transpose(
                                out=tp[:], in_=hN[:, ts(cc, 128)],
                                identity=ident[:])
                            nc.vector.tensor_copy(out=rows[:, cc, :], in_=tp[:])
                            if li < N_LAYERS - 1 and USE_AG:
                                nc.scalar.activation(
                                    out=rows_bf[:, cc, :], in_=tp[:],
                                    func=mybir.ActivationFunctionType.Copy,
                                )
                        xw_cur[0] = None

                    for gg in range(NGG):
                        mt = {}
                        for b in (0, 1):
                            nch = int(p.NCH[gg, b])
                            if nch == 0:
                                mt[b] = None
                                continue
                            m = msgp.tile([128, p.MSG_CH, 256], BF16,
                                          tag=f"msg{b}")
                            src_ap = bucket_view(table, b)
                            base = int(p.chbase[gg, b])
                            for k0 in range(0, nch, SUB_CH):
                                k1 = min(nch, k0 + SUB_CH)
                                nc.gpsimd.dma_gather(
                                    out_ap=m[:, k0:k1, :],
                                    in_ap=src_ap,
                                    idxs_ap=idx_sb[b][:, (base + k0) * 8:
                                                      (base + k1) * 8],
                                    num_idxs=(k1 - k0) * 128,
                                    num_idxs_reg=(k1 - k0) * 128,
                                    elem_size=256,
                                    elem_step=128,
                                    single_packet=False,
                                    queue_num=next_q(),
                                )
                            mt[b] = m

                        for c in range(gg * CPG, (gg + 1) * CPG):
                            mms = p.cell_mms[c]
                            xt, c0, clen = xw_for_cell(c)
                            colsl = ts(c - c0, 128)
                            if not mms:
                                nc.vector.tensor_copy(
                                    out=xt[:, colsl], in_=hT[:, ts(c, 128)]
                                )
                                finish_cell(c)
                                continue
                            agg = ps_agg.tile([128, W], F32, tag="agg")
                            nmm = len(mms)
                            for j, (b, k, mm) in enumerate(mms):
                                nc.tensor.matmul(
                                    out=agg[:],
                                    lhsT=mt[b][:, k, 0:128],
                                    rhs=get_oh(mm),
                                    start=(j == 0), stop=(j == nmm - 1),
                                )
                            nc.vector.tensor_tensor(
                                out=xt[:, colsl], in0=agg[:],
                                in1=hT[:, ts(c, 128)],
                                op=mybir.AluOpType.add,
                            )
                            finish_cell(c)

                        # AllGather per quarter as soon as its rows are final
                        if li < N_LAYERS - 1 and USE_AG and gg % (NGG // NQTR) \
                                == (NGG // NQTR) - 1:
                            qt = gg // (NGG // NQTR)
                            nc.sync.dma_start(
                                out=agin[qt].rearrange("(t p) d -> p t d",
                                                       p=128),
                                in_=rows_bf[:, qt * CPQ:(qt + 1) * CPQ, :],
                            )
                            nc.gpsimd.collective_compute(
                                "AllGather",
                                mybir.AluOpType.bypass,
                                ins=[agin[qt][:]],
                                outs=[table_q_rows(tables[li + 1], qt)],
                                replica_groups=[list(range(M))],
                            )
                    hT, hN = hN, hT

                # ---- head
                pool3 = ps_pool.tile([128, GPC], F32, tag="pool")
                pool_tiles(lambda t: rows[:, t, :], NT, pooloh, pool3)
                hg = smp.tile([128, GPC], F32, tag="hg")
                nc.vector.tensor_copy(out=hg[:], in_=pool3[:])
                h1_ps = ps_tr.tile([128, GPC], F32, tag="tr")
                nc.tensor.matmul(out=h1_ps[:], lhsT=fc1W[:], rhs=hg[:],
                                 start=True, stop=True)
                hfc = smp.tile([128, GPC], F32, tag="hfc")
                nc.scalar.activation(
                    out=hfc[:], in_=h1_ps[:],
                    func=mybir.ActivationFunctionType.Relu, bias=fc1b[:, 0:1],
                )
                q_ps = ps_tr.tile([NA, GPC], F32, tag="tr")
                nc.tensor.matmul(out=q_ps[:], lhsT=fc2W[:], rhs=hfc[:],
                                 start=True, stop=True)
                q_sb = smp.tile([NA, GPC], F32, tag="qsb")
                nc.vector.tensor_scalar(
                    out=q_sb[:], in0=q_ps[:], scalar1=fc2b[:, 0:1],
                    scalar2=None, op0=mybir.AluOpType.add,
                )
                nc.sync.dma_start(out=q_d[:], in_=q_sb[:])

    nc.compile()
    return nc


# ---------------------------------------------------------------- driver


_CACHE = {}


def _in_maps(p, inputs_np):
    maps = []
    for d in range(M):
        maps.append({
            "table0": np.asarray(p.table0),
            "rows0": p.rows0[d],
            "initrows": p.initrows[d],
            "leadrows": p.leadrows[d],
            "pooloh": p.pooloh[d],
            "poolohi": p.pooloh_init[d],
            "poolohl": p.pooloh_lead[d],
            "dstcol": p.dstcol[d],
            "idx0": p.idx_tiles[d][0],
            "idx1": p.idx_tiles[d][1],
            "iota_oh": p.iota_oh,
            "W1": inputs_np["W1"], "W2": inputs_np["W2"], "W3": inputs_np["W3"],
            "b1": inputs_np["b1"].reshape(D, 1),
            "b2": inputs_np["b2"].reshape(D, 1),
            "b3": inputs_np["b3"].reshape(D, 1),
            "fc1W": inputs_np["fc1_W"],
            "fc1b": inputs_np["fc1_b"].reshape(D, 1),
            "fc2W": inputs_np["fc2_W"],
            "fc2b": inputs_np["fc2_b"].reshape(NA, 1),
        })
    return maps


def _get_program(inputs_np, reps=REPS):
    key = ("prog", reps)
    if key not in _CACHE:
        if "plan" not in _CACHE:
            _CACHE["plan"] = build_plan(
                inputs_np["inputs"], inputs_np["src"], inputs_np["dst"],
                inputs_np["graph_ids"], inputs_np["init_feats"],
                inputs_np["init_graph_ids"], inputs_np["lead_feats"],
                inputs_np["lead_graph_ids"],
            )
        p = _CACHE["plan"]
        nc = build_bass(p, reps=reps)
        _CACHE[key] = (p, nc)
    return _CACHE[key]


def _run(inputs_np, trace=False):
    p, nc = _get_program(inputs_np)
    res = run_bass_kernel_spmd(nc, _in_maps(p, inputs_np), list(range(M)),
                               trace=trace)
    out = np.zeros((NG, NA), np.float32)
    for d in range(M):
        out[d * GPC:(d + 1) * GPC] = res.results[d]["q"].T
    return out, res


def kernel(**inputs):
    inputs_np = {k: np.asarray(v) for k, v in inputs.items()}
    out, _ = _run(inputs_np, trace=False)
    return out
